# revision 47
# baseline (speedup 1.0000x reference)
"""NeRF hierarchical sampling + positional encoding kernel for Trainium2.

Full inputs -> shard rays across 8 cores (data-parallel over the ray axis)
-> one Bass program per core (8 tiles of 128 rays) -> full output.

O(N+S) per-ray sampler (no dense S*N compare), all fp32-exact vs reference:
  pdf/cdf prep, then per-interval slope/intercept:
    Mstar_j = (bins_{j+1}-bins_j)/denomsafe_j/S,  A_j = bins_j - cdf_j*M_j
  so sample_s = su_s*Mstar[k_s] + A[k_s] with su_s = s + u_rand, and
  k_s = searchsorted(cdf, u)-1.
  Ranks r_j = #{s: su_s < S*cdf_j} come from r_j = c_j + [su[c_j] < y_j]
  (c_j = floor clamp of y_j = S*cdf_j): su[c_j] is gathered by gpsimd
  local_scatter (per-partition indices) + DVE fill-forward scans; the
  inverse of the sorted integer sequence c is itself one scatter + scan.
  A[k]/Mstar[k] are gathered the same way, scattering each fp32 value as
  two int16 bit-halves (exact) at deduplicated rank positions.
  Positional encoding: one magic-round + Cody-Waite range reduction at
  deg 0, then one add_range_wrap halving per degree; sin and cos(=shifted
  sin) evaluated by the scalar engine's Sin activation writing straight
  into the staging tile; view encode batched over all degrees per tile.
Engines: DVE does the vector work, gpsimd only local_scatters (library 7),
ACT does all transcendentals + the broadcast view-block copy, sync issues
contiguous DMAs (output is DMA-bandwidth-floor ~154us/core).
"""

import os
import sys

for _p in ("/opt/trn_rl_repo", "/root/.axon_site/_ro/trn_rl_repo"):
    if os.path.isdir(_p) and _p not in sys.path:
        sys.path.insert(0, _p)

import numpy as np

import concourse.bass as bass
import concourse.bacc as bacc
import concourse.mybir as mybir
from concourse import tile

F32 = mybir.dt.float32
BF16 = mybir.dt.bfloat16
I32 = mybir.dt.int32
I16 = mybir.dt.int16
ALU = mybir.AluOpType
ACTF = mybir.ActivationFunctionType

R, N, S = 8192, 128, 128
NCORES = 8
RC = R // NCORES          # rays per core
NT = RC // 128            # ray tiles per core (128 rays each)
DEG = 10
EPS = 1e-5
CH = 120                  # output channels per sample
OUTW = S * CH             # flattened output row per ray

PI = float(np.float32(np.pi))
TWO_PI = 6.283185307179586
INV_2PI = float(np.float32(1.0 / TWO_PI))
MAGIC = float(np.float32(1.5 * 2**23))  # round-to-int magic constant
# Cody-Waite split of 2*pi (fallback encode path)
CW1 = 6.25
CW2 = 0.033203125
CW3 = float(np.float32(TWO_PI - CW1 - CW2))

HALF_S = 64               # encode/staging processed in s-halves
BIG = 1.0e9               # pad sentinel for compare columns

def _emit_encode_deg(nc, stg3, rs, l, work):
    """Emit sin/cos for degree l from range-reduced rs (s-major [128, 3*S]).

    rs holds x - k*2pi/2^l with |rs| <= pi/2^l; ACT applies scale 2^l.
    """
    sc = float(2.0 ** l)
    rs_3 = rs[:, :].rearrange("p (s k) -> p s k", k=3)
    sin_dst = stg3[:, :, 3 * l:3 * l + 3]
    nc.scalar.activation(sin_dst, rs_3, ACTF.Sin, bias=0.0, scale=sc)
    ws = work.tile([128, 3 * S], F32, tag="ws")
    nc.vector.add_range_wrap(ws[:, :], rs[:, :], (TWO_PI / 4.0) / sc,
                             (TWO_PI / 2.0) / sc, TWO_PI / sc)
    ws_3 = ws[:, :].rearrange("p (s k) -> p s k", k=3)
    cos_dst = stg3[:, :, 30 + 3 * l:30 + 3 * l + 3]
    nc.scalar.activation(cos_dst, ws_3, ACTF.Sin, bias=0.0, scale=sc)


def _emit_core_kernel(nc):
    """Emit the whole per-core program under a TileContext."""
    org_h = nc.dram_tensor("origins", [RC, 3], F32, kind="ExternalInput")
    dir_h = nc.dram_tensor("directions", [RC, 3], F32, kind="ExternalInput")
    bins_h = nc.dram_tensor("bins", [RC, N], F32, kind="ExternalInput")
    w_h = nc.dram_tensor("weights", [RC, N], F32, kind="ExternalInput")
    ur_h = nc.dram_tensor("u_rand", [RC, S], F32, kind="ExternalInput")
    out_h = nc.dram_tensor("out", [RC, OUTW], F32, kind="ExternalOutput")

    with tile.TileContext(nc) as tc:
        with (
            tc.tile_pool(name="io", bufs=3) as io,
            tc.tile_pool(name="cmp", bufs=2) as cmp_pool,
            tc.tile_pool(name="stage", bufs=2) as stage_pool,
            tc.tile_pool(name="work", bufs=2) as work,
            tc.tile_pool(name="const", bufs=1) as cpool,
        ):
            # --- constants (once) ---
            iota_i = cpool.tile([128, S], I32)
            nc.gpsimd.iota(iota_i[:, :], pattern=[[1, S]], base=0,
                           channel_multiplier=0)
            # all later gpsimd work is local_scatter (library 7)
            from concourse import library_config
            nc.gpsimd.load_library(library_config.local_scatter)
            iota_f = cpool.tile([128, S], F32)
            nc.vector.tensor_copy(iota_f[:, :], iota_i[:, :])
            ones_t = cpool.tile([128, S], F32)
            nc.vector.memset(ones_t[:, :], 1.0)
            ones16 = cpool.tile([128, N], I16)
            nc.vector.memset(ones16[:, :], 1)
            # iotaj16[j] = j+2 (int16) D-scatter payload: the fill-forward
            # then yields q'_v + 1 directly, so idxw = fq*pv - 1
            iotaj16 = cpool.tile([128, N], I16)
            iotaj_f = work.tile([128, N], F32, tag="iotajf")
            nc.vector.tensor_scalar(iotaj_f[:, :], iota_f[:, :], 2.0, None,
                                    ALU.add)
            nc.vector.tensor_copy(iotaj16[:, :], iotaj_f[:, :])
            # per-column scales 2^l for the view encode: [128, 30]
            sc30 = cpool.tile([128, DEG * 3], F32)
            for l in range(DEG):
                nc.vector.memset(sc30[:, 3 * l:3 * l + 3], float(2.0 ** l))

            for t in range(NT):
                r0 = t * 128
                bins_t = io.tile_from(bins_h[r0:r0 + 128, :])
                w_t = io.tile_from(w_h[r0:r0 + 128, :])
                ur_t = io.tile_from(ur_h[r0:r0 + 128, :])
                org_t = io.tile_from(org_h[r0:r0 + 128, :])
                dir_t = io.tile_from(dir_h[r0:r0 + 128, :])

                # ---- view encode, batched over degs: vt [128, 60] ----
                vt = work.tile([128, 2 * DEG * 3], F32, tag="vt")
                zd = work.tile([128, DEG * 3], F32, tag="zd")
                dir_b = dir_t[:, :].unsqueeze(1).broadcast_to((128, DEG, 3))
                zd3 = zd[:, :].rearrange("p (l k) -> p l k", k=3)
                sc30_3 = sc30[:, :].rearrange("p (l k) -> p l k", k=3)
                nc.vector.tensor_tensor(zd3, dir_b, sc30_3, ALU.mult)
                tv = work.tile([128, DEG * 3], F32, tag="tv")
                nc.vector.tensor_scalar(tv[:, :], zd[:, :], INV_2PI, MAGIC,
                                        ALU.mult, ALU.add)
                nc.vector.tensor_scalar(tv[:, :], tv[:, :], MAGIC, None,
                                        ALU.subtract)
                rv = work.tile([128, DEG * 3], F32, tag="rv")
                nc.vector.scalar_tensor_tensor(rv[:, :], tv[:, :], -TWO_PI,
                                               zd[:, :], ALU.mult, ALU.add)
                nc.scalar.activation(vt[:, 0:DEG * 3], rv[:, :], ACTF.Sin,
                                     bias=0.0, scale=1.0)
                rvc = work.tile([128, DEG * 3], F32, tag="rvc")
                nc.vector.add_range_wrap(rvc[:, :], rv[:, :], TWO_PI / 4.0,
                                         TWO_PI / 2.0, TWO_PI)
                nc.scalar.activation(vt[:, DEG * 3:2 * DEG * 3], rvc[:, :],
                                     ACTF.Sin, bias=0.0, scale=1.0)

                # ---- pdf / cdf  (matches reference op order) ----
                wsum = work.tile([128, 1], F32, tag="wsum")
                nc.vector.tensor_reduce(wsum[:, :], w_t[:, 0:N - 1],
                                        axis=mybir.AxisListType.X, op=ALU.add)
                pad = work.tile([128, 1], F32, tag="pad")
                nc.vector.tensor_scalar(pad[:, :], wsum[:, :], -1.0, EPS,
                                        ALU.mult, ALU.add)
                nc.vector.tensor_scalar(pad[:, :], pad[:, :], 0.0, None, ALU.max)
                wsum2 = work.tile([128, 1], F32, tag="wsum2")
                nc.vector.tensor_tensor(wsum2[:, :], wsum[:, :], pad[:, :], ALU.add)
                rws = work.tile([128, 1], F32, tag="rws")
                nc.vector.reciprocal(rws[:, :], wsum2[:, :])
                padc = work.tile([128, 1], F32, tag="padc")
                nc.vector.tensor_scalar(padc[:, :], pad[:, :], 1.0 / (N - 1), None,
                                        ALU.mult)
                pdf = work.tile([128, N - 1], F32, tag="pdf")
                nc.vector.scalar_tensor_tensor(
                    pdf[:, :], w_t[:, 0:N - 1], padc[:, 0:1],
                    rws[:, 0:1].broadcast_to((128, N - 1)), ALU.add, ALU.mult)

                cdf = work.tile([128, N], F32, tag="cdf")
                nc.vector.memset(cdf[:, 0:1], 0.0)
                nc.vector.memset(cdf[:, N - 1:N], 1.0)
                cs = work.tile([128, N - 2], F32, tag="cs")
                nc.vector.tensor_tensor_scan(cs[:, :], ones_t[:, 0:N - 2],
                                             pdf[:, 0:N - 2], 0.0,
                                             ALU.mult, ALU.add)
                nc.vector.tensor_scalar(cdf[:, 1:N - 1], cs[:, :], 1.0, None,
                                        ALU.min)

                # ---- per-interval slope/intercept (j = 0..126) ----
                d0 = work.tile([128, N - 1], F32, tag="d0")
                nc.vector.tensor_tensor(d0[:, :], cdf[:, 1:N], cdf[:, 0:N - 1],
                                        ALU.subtract)
                db = work.tile([128, N - 1], F32, tag="db")
                nc.vector.tensor_tensor(db[:, :], bins_t[:, 1:N],
                                        bins_t[:, 0:N - 1], ALU.subtract)
                maskE = work.tile([128, N - 1], mybir.dt.uint8, tag="maskE")
                nc.vector.tensor_scalar(maskE[:, :], d0[:, :], EPS, None,
                                        ALU.is_lt)
                dsafe = work.tile([128, N - 1], F32, tag="dsafe")
                nc.vector.select(dsafe[:, :], maskE[:, :], ones_t[:, 0:N - 1],
                                 d0[:, :])
                # M = db / dsafe ; Mstar = M / S ; A = bins - cdf * M
                rdsafe = work.tile([128, N - 1], F32, tag="rdsafe")
                nc.vector.reciprocal(rdsafe[:, :], dsafe[:, :])
                m_t = work.tile([128, N - 1], F32, tag="m_t")
                nc.vector.tensor_tensor(m_t[:, :], db[:, :], rdsafe[:, :],
                                        ALU.mult)
                ms_t = work.tile([128, N], F32, tag="ms_t")
                nc.vector.tensor_scalar(ms_t[:, 0:N - 1], m_t[:, :], 1.0 / S,
                                        None, ALU.mult)
                nc.vector.memset(ms_t[:, N - 1:N], 0.0)
                cm = work.tile([128, N - 1], F32, tag="cm")
                nc.vector.tensor_tensor(cm[:, :], cdf[:, 0:N - 1], m_t[:, :],
                                        ALU.mult)
                a_t = work.tile([128, N], F32, tag="a_t")
                nc.vector.tensor_tensor(a_t[:, 0:N - 1], bins_t[:, 0:N - 1],
                                        cm[:, :], ALU.subtract)
                nc.vector.memset(a_t[:, N - 1:N], 0.0)

                # ---- compare inputs ----
                su = work.tile([128, S], F32, tag="su")
                nc.vector.tensor_tensor(su[:, :], iota_f[:, :], ur_t[:, :], ALU.add)
                y2 = work.tile([128, N - 1], F32, tag="y2")
                nc.vector.tensor_scalar(y2[:, :], cdf[:, 1:N], float(S), None,
                                        ALU.mult)

                # ---- O(N+S) ranks, no dense compare:
                #   r2[j] = #{s : su_s < y_j} = c_j + [su[c_j] < y_j],
                #   c_j = min(floor(y_j), 127).  su[c_j] is a scatter-scan
                #   gather: the inverse of sorted-int c needs no compare.
                cr = work.tile([128, N - 1], F32, tag="cr")
                nc.vector.tensor_scalar(cr[:, :], y2[:, :], 1.0, MAGIC,
                                        ALU.mult, ALU.add)
                nc.vector.tensor_scalar(cr[:, :], cr[:, :], MAGIC, None,
                                        ALU.subtract)
                cgt = work.tile([128, N - 1], F32, tag="cgt")
                nc.vector.tensor_tensor(cgt[:, :], cr[:, :], y2[:, :], ALU.is_gt)
                cfl = work.tile([128, N - 1], F32, tag="cfl")
                nc.vector.tensor_tensor(cfl[:, :], cr[:, :], cgt[:, :],
                                        ALU.subtract)
                nc.vector.tensor_scalar(cfl[:, :], cfl[:, :], float(S - 1),
                                        None, ALU.min)
                # D-scatter: place (j+1) at cell c_j, keep largest j per value
                kdx = work.tile([128, N - 1], F32, tag="kdx")
                nc.vector.tensor_tensor(kdx[:, 0:N - 2], cfl[:, 0:N - 2],
                                        cfl[:, 1:N - 1], ALU.is_lt)
                nc.vector.memset(kdx[:, N - 2:N - 1], 1.0)
                kt1 = work.tile([128, N - 1], F32, tag="kt1")
                nc.vector.tensor_tensor(kt1[:, :], cfl[:, :], kdx[:, :],
                                        ALU.mult)
                kt2 = work.tile([128, N - 1], F32, tag="kt2")
                nc.vector.tensor_scalar(kt2[:, :], kdx[:, :], 1.0, None,
                                        ALU.subtract)
                idxcf = work.tile([128, N - 1], F32, tag="idxcf")
                nc.vector.tensor_tensor(idxcf[:, :], kt1[:, :], kt2[:, :],
                                        ALU.add)
                idxc16 = work.tile([128, N], I16, tag="idxc16")
                nc.vector.tensor_copy(idxc16[:, 0:N - 1], idxcf[:, :])
                nc.vector.memset(idxc16[:, N - 1:N], -1)
                d16 = work.tile([128, S], I16, tag="d16")
                nc.gpsimd.local_scatter(d16[:, :], iotaj16[:, :], idxc16[:, :],
                                        channels=128, num_elems=S, num_idxs=N)
                # F[v] = #{j : c_j <= v} by fill-forward; q'_v = F[v-1]
                aD = work.tile([128, S], F32, tag="aD")
                nc.vector.tensor_scalar(aD[:, :], d16[:, :], 0.0, None,
                                        ALU.is_equal)
                fq = work.tile([128, S + 1], F32, tag="fq")
                nc.vector.memset(fq[:, 0:1], 0.0)
                nc.vector.tensor_tensor_scan(fq[:, 1:S + 1], aD[:, :],
                                             d16[:, :], 0.0, ALU.mult, ALU.add)
                pv = work.tile([128, S], F32, tag="pv")
                nc.vector.tensor_scalar(pv[:, :], d16[:, :], 0.0, None,
                                        ALU.is_gt)
                wt1 = work.tile([128, S], F32, tag="wt1")
                nc.vector.tensor_tensor(wt1[:, :], fq[:, 0:S], pv[:, :],
                                        ALU.mult)
                idxwf = work.tile([128, S], F32, tag="idxwf")
                nc.vector.tensor_scalar(idxwf[:, :], wt1[:, :], 1.0, None,
                                        ALU.subtract)
                idxw16 = work.tile([128, S], I16, tag="idxw16")
                nc.vector.tensor_copy(idxw16[:, :], idxwf[:, :])
                occw = work.tile([128, S], I16, tag="occw")
                nc.gpsimd.local_scatter(occw[:, :], ones16[:, :], idxw16[:, :],
                                        channels=128, num_elems=S, num_idxs=S)
                aW = work.tile([128, S], F32, tag="aW")
                nc.vector.tensor_scalar(aW[:, :], occw[:, :], -1.0, 1.0,
                                        ALU.mult, ALU.add)
                # gather su[c_j] into W cells via int16-halves scatter + scan
                su16 = su[:, :].bitcast(I16).rearrange(
                    "p (s two) -> p s two", two=2)
                w16 = work.tile([128, 2 * S], I16, tag="w16")
                w16v = w16[:, :].rearrange("p (j two) -> p j two", two=2)
                for half in range(2):
                    shalf = work.tile([128, S], I16, tag=f"suh{half}")
                    nc.vector.tensor_copy(shalf[:, :], su16[:, :, half])
                    wsct = work.tile([128, S], I16, tag=f"wsct{half}")
                    nc.gpsimd.local_scatter(wsct[:, :], shalf[:, :],
                                            idxw16[:, :], channels=128,
                                            num_elems=S, num_idxs=S)
                    nc.vector.tensor_tensor_scan(w16v[:, :, half], aW[:, :],
                                                 wsct[:, :], 0.0, ALU.mult,
                                                 ALU.add)
                wg = w16[:, :].bitcast(F32)
                # t_j = [su[c_j] < y_j];  r2[j] = c_j + t_j
                tj = work.tile([128, N - 1], F32, tag="tj")
                nc.vector.tensor_tensor(tj[:, :], wg[:, 0:N - 1], y2[:, :],
                                        ALU.is_lt)
                r2 = work.tile([128, N], F32, tag="r2")
                nc.vector.memset(r2[:, 0:1], 0.0)
                nc.vector.tensor_tensor(r2[:, 1:N], cfl[:, :], tj[:, :],
                                        ALU.add)

                # scatter position for interval j (j = 0..126), keeping only
                # the largest j at each rank: idx_j = r2[j] iff r2[j] < r2[j+1]
                km = work.tile([128, N - 1], F32, tag="km")
                nc.vector.tensor_tensor(km[:, :], r2[:, 0:N - 1], r2[:, 1:N],
                                        ALU.is_lt)
                kt = work.tile([128, N - 1], F32, tag="kt")
                nc.vector.tensor_tensor(kt[:, :], r2[:, 0:N - 1], km[:, :],
                                        ALU.mult)
                km1 = work.tile([128, N - 1], F32, tag="km1")
                nc.vector.tensor_scalar(km1[:, :], km[:, :], 1.0, None,
                                        ALU.subtract)
                idxf = work.tile([128, N - 1], F32, tag="idxf")
                nc.vector.tensor_tensor(idxf[:, :], kt[:, :], km1[:, :], ALU.add)
                idx16 = work.tile([128, N], I16, tag="idx16")
                nc.vector.tensor_copy(idx16[:, 0:N - 1], idxf[:, :])
                nc.vector.memset(idx16[:, N - 1:N], -1)

                # occupancy scatter + fill-forward mask
                occ16 = work.tile([128, S], I16, tag="occ16")
                nc.gpsimd.local_scatter(occ16[:, :], ones16[:, :], idx16[:, :],
                                        channels=128, num_elems=S, num_idxs=N)
                amask = work.tile([128, S], I16, tag="amask")
                nc.vector.tensor_scalar(amask[:, :], occ16[:, :], -1.0, 1.0,
                                        ALU.mult, ALU.add)

                # exact f32 gathers A[k_s], Mstar[k_s]: scatter the two int16
                # halves of each value, fill-forward scan, reinterleave
                gath = {}
                for name, vsrc in (("A", a_t), ("M", ms_t)):
                    v16 = vsrc[:, :].bitcast(I16).rearrange(
                        "p (j two) -> p j two", two=2)
                    g16 = work.tile([128, 2 * S], I16, tag=f"g16{name}")
                    g16v = g16[:, :].rearrange("p (s two) -> p s two", two=2)
                    for half in range(2):
                        hsrc = work.tile([128, N], I16, tag=f"h{name}{half}")
                        nc.vector.tensor_copy(hsrc[:, :], v16[:, :, half])
                        sct = work.tile([128, S], I16, tag=f"sct{name}{half}")
                        nc.gpsimd.local_scatter(sct[:, :], hsrc[:, :],
                                                idx16[:, :], channels=128,
                                                num_elems=S, num_idxs=N)
                        nc.vector.tensor_tensor_scan(g16v[:, :, half],
                                                     amask[:, :], sct[:, :],
                                                     0.0, ALU.mult, ALU.add)
                    gath[name] = g16[:, :].bitcast(F32)

                # ---- interpolation: smp = su*Mstar[k] + A[k] ----
                tmp = work.tile([128, S], F32, tag="tmp")
                nc.vector.tensor_tensor(tmp[:, :], su[:, :], gath["M"], ALU.mult)
                smp = work.tile([128, S], F32, tag="smp")
                nc.vector.tensor_tensor(smp[:, :], tmp[:, :], gath["A"], ALU.add)

                # ---- points, s-major interleaved [128, S*3] ----
                pts = work.tile([128, 3 * S], F32, tag="pts")
                pts_k = pts[:, :].rearrange("p (s k) -> p k s", k=3)
                for k in range(3):
                    nc.vector.scalar_tensor_tensor(
                        pts_k[:, k, :], smp[:, :], dir_t[:, k:k + 1],
                        org_t[:, k:k + 1].broadcast_to((128, S)),
                        ALU.mult, ALU.add)
                yb = work.tile([128, 3 * S], F32, tag="yb")
                nc.vector.tensor_scalar(yb[:, :], pts[:, :], INV_2PI, None,
                                        ALU.mult)

                # ---- positional encodes + staging + store, per s-half ----
                stg = stage_pool.tile([128, S * CH], F32, tag="stg")
                stg3 = stg[:, :].rearrange("p (s c) -> p s c", c=CH)
                # range-reduce once at deg 0 (magic round + Cody-Waite),
                # then halve the range per degree with one wrap each
                t1 = work.tile([128, 3 * S], F32, tag="t1")
                nc.vector.tensor_scalar(t1[:, :], yb[:, :], 1.0, MAGIC,
                                        ALU.mult, ALU.add)
                nc.vector.tensor_scalar(t1[:, :], t1[:, :], MAGIC, None,
                                        ALU.subtract)
                rs = work.tile([128, 3 * S], F32, tag="rs")
                nc.vector.cody_waite_cascade(rs[:, :], pts[:, :], t1[:, :],
                                             CW1, CW2, CW3)
                for l in range(DEG):
                    if l > 0:
                        sc = float(2.0 ** l)
                        rs_new = work.tile([128, 3 * S], F32, tag="rs")
                        nc.vector.add_range_wrap(rs_new[:, :], rs[:, :], 0.0,
                                                 (TWO_PI / 2.0) / sc,
                                                 TWO_PI / sc)
                        rs = rs_new
                    _emit_encode_deg(nc, stg3, rs, l, work)
                # view block: broadcast [128, 60] over all s
                vin = vt[:, :].unsqueeze(1).broadcast_to((128, S, 60))
                nc.scalar.copy(stg3[:, :, 60:120], vin)
                nc.sync.dma_start(out_h[r0:r0 + 128, :], stg[:, :])
    return nc


_NC_CACHE = {}


def _get_nc():
    if "nc" not in _NC_CACHE:
        nc = bacc.Bacc('TRN2', target_bir_lowering=False)
        _emit_core_kernel(nc)
        nc.compile()
        _NC_CACHE["nc"] = nc
    return _NC_CACHE["nc"]


def _shard(inputs):
    in_maps = []
    for c in range(NCORES):
        sl = slice(c * RC, (c + 1) * RC)
        in_maps.append({
            "origins": np.ascontiguousarray(inputs["origins"][sl]),
            "directions": np.ascontiguousarray(inputs["directions"][sl]),
            "bins": np.ascontiguousarray(inputs["bins"][sl]),
            "weights": np.ascontiguousarray(inputs["weights"][sl]),
            "u_rand": np.ascontiguousarray(inputs["u_rand"][sl]),
        })
    return in_maps


LAST_EXEC_NS = None
LAST_TRACE_PATH = None
LAST_RES = None


def kernel(**inputs):
    global LAST_EXEC_NS, LAST_TRACE_PATH, LAST_RES
    from concourse.bass_utils import run_bass_kernel_spmd
    nc = _get_nc()
    in_maps = _shard(inputs)
    trace = bool(os.environ.get("BASS_TRACE"))
    res = run_bass_kernel_spmd(nc, in_maps, core_ids=list(range(NCORES)),
                               trace=trace)
    if trace:
        LAST_RES = res
        LAST_EXEC_NS = res.exec_time_ns
        print("HW exec_time_ns:", res.exec_time_ns,
              "mean:", res.mean_exec_time_ns)
        if res.instructions_and_trace:
            LAST_TRACE_PATH = res.instructions_and_trace[1]
            print("trace path:", res.instructions_and_trace[1])
    parts = [res.results[c]["out"].reshape(RC, S, CH) for c in range(NCORES)]
    return np.concatenate(parts, axis=0).astype(np.float32)


def simulate_one_core(core_inputs):
    """CoreSim path for numerics debugging (no hardware)."""
    from concourse.bass_interp import CoreSim
    nc = bacc.Bacc('TRN2', target_bir_lowering=False)
    _emit_core_kernel(nc)
    nc.compile()
    sim = CoreSim(nc, require_finite=False, require_nnan=False)
    if sim.instruction_executor is not None:
        sim.instruction_executor.ignore_data_errors = True
    for k, v in core_inputs.items():
        sim.tensor(k)[:] = v
    sim.simulate()
    return np.array(sim.tensor("out")).reshape(RC, S, CH)


# revision 49
# speedup vs baseline: 1.0218x; 1.0218x over previous
"""NeRF hierarchical sampling + positional encoding kernel for Trainium2.

Full inputs -> shard rays across 8 cores (data-parallel over the ray axis)
-> one Bass program per core (8 tiles of 128 rays) -> full output.

O(N+S) per-ray sampler (no dense S*N compare), fp32-exact vs reference:
  pdf/cdf prep, then per-interval slope/intercept:
    Mstar_j = (bins_{j+1}-bins_j)/denomsafe_j/S,  A_j = bins_j - cdf_j*M_j
  so sample_s = su_s*Mstar[k_s] + A[k_s] with su_s = s + u_rand, and
  k_s = searchsorted(cdf, u)-1.
  Ranks r_j = #{s: su_s < S*cdf_j} come from r_j = c_j + [su[c_j] < y_j]
  (c_j = clamped floor of y_j = S*cdf_j): su[c_j] is gathered by gpsimd
  local_scatter (per-partition indices) + DVE fill-forward scans; the
  inverse of the sorted integer sequence c is itself one scatter + scan.
  A[k]/Mstar[k] are gathered the same way, scattering each fp32 value as
  two int16 bit-halves (exact) at deduplicated rank positions.
  Positional encoding: one magic-round + Cody-Waite range reduction at
  deg 0, then one add_range_wrap halving per degree; sin and cos(=shifted
  sin) evaluated by the scalar engine's Sin activation writing straight
  into the staging tile; view encode batched over all degrees per tile.
Engines: DVE does the vector work, gpsimd only local_scatters (library 7),
ACT does all transcendentals + the broadcast view-block copy, sync issues
contiguous DMAs (output is DMA-bandwidth-floor ~154us/core).
"""

import os
import sys

for _p in ("/opt/trn_rl_repo", "/root/.axon_site/_ro/trn_rl_repo"):
    if os.path.isdir(_p) and _p not in sys.path:
        sys.path.insert(0, _p)

import numpy as np

import concourse.bass as bass
import concourse.bacc as bacc
import concourse.mybir as mybir
from concourse import tile

F32 = mybir.dt.float32
BF16 = mybir.dt.bfloat16
I32 = mybir.dt.int32
I16 = mybir.dt.int16
ALU = mybir.AluOpType
ACTF = mybir.ActivationFunctionType

R, N, S = 8192, 128, 128
NCORES = 8
RC = R // NCORES          # rays per core
NT = RC // 128            # ray tiles per core (128 rays each)
DEG = 10
EPS = 1e-5
CH = 120                  # output channels per sample
OUTW = S * CH             # flattened output row per ray

PI = float(np.float32(np.pi))
TWO_PI = 6.283185307179586
INV_2PI = float(np.float32(1.0 / TWO_PI))
MAGIC = float(np.float32(1.5 * 2**23))  # round-to-int magic constant
# Cody-Waite split of 2*pi (fallback encode path)
CW1 = 6.25
CW2 = 0.033203125
CW3 = float(np.float32(TWO_PI - CW1 - CW2))

HALF_S = 64               # encode/staging processed in s-halves
BIG = 1.0e9               # pad sentinel for compare columns

def _emit_encode_deg(nc, stg3, rs, l, work):
    """Emit sin/cos for degree l from range-reduced rs (s-major [128, 3*S]).

    rs holds x - k*2pi/2^l with |rs| <= pi/2^l; ACT applies scale 2^l.
    """
    sc = float(2.0 ** l)
    rs_3 = rs[:, :].rearrange("p (s k) -> p s k", k=3)
    sin_dst = stg3[:, :, 3 * l:3 * l + 3]
    nc.scalar.activation(sin_dst, rs_3, ACTF.Sin, bias=0.0, scale=sc)
    ws = work.tile([128, 3 * S], F32, tag="ws")
    nc.vector.add_range_wrap(ws[:, :], rs[:, :], (TWO_PI / 4.0) / sc,
                             (TWO_PI / 2.0) / sc, TWO_PI / sc)
    ws_3 = ws[:, :].rearrange("p (s k) -> p s k", k=3)
    cos_dst = stg3[:, :, 30 + 3 * l:30 + 3 * l + 3]
    nc.scalar.activation(cos_dst, ws_3, ACTF.Sin, bias=0.0, scale=sc)


def _emit_core_kernel(nc):
    """Emit the whole per-core program under a TileContext."""
    org_h = nc.dram_tensor("origins", [RC, 3], F32, kind="ExternalInput")
    dir_h = nc.dram_tensor("directions", [RC, 3], F32, kind="ExternalInput")
    bins_h = nc.dram_tensor("bins", [RC, N], F32, kind="ExternalInput")
    w_h = nc.dram_tensor("weights", [RC, N], F32, kind="ExternalInput")
    ur_h = nc.dram_tensor("u_rand", [RC, S], F32, kind="ExternalInput")
    out_h = nc.dram_tensor("out", [RC, OUTW], F32, kind="ExternalOutput")

    with tile.TileContext(nc) as tc:
        with (
            tc.tile_pool(name="io", bufs=3) as io,
            tc.tile_pool(name="cmp", bufs=2) as cmp_pool,
            tc.tile_pool(name="stage", bufs=2) as stage_pool,
            tc.tile_pool(name="work", bufs=2) as work,
            tc.tile_pool(name="const", bufs=1) as cpool,
        ):
            # --- constants (once) ---
            iota_i = cpool.tile([128, S], I32)
            nc.gpsimd.iota(iota_i[:, :], pattern=[[1, S]], base=0,
                           channel_multiplier=0)
            # all later gpsimd work is local_scatter (library 7)
            from concourse import library_config
            nc.gpsimd.load_library(library_config.local_scatter)
            iota_f = cpool.tile([128, S], F32)
            nc.vector.tensor_copy(iota_f[:, :], iota_i[:, :])
            ones_t = cpool.tile([128, S], F32)
            nc.vector.memset(ones_t[:, :], 1.0)
            ones16 = cpool.tile([128, N], I16)
            nc.vector.memset(ones16[:, :], 1)
            # iotaj16[j] = j+2 (int16) D-scatter payload: the fill-forward
            # then yields q'_v + 1 directly, so idxw = fq*pv - 1
            iotaj16 = cpool.tile([128, N], I16)
            iotaj_f = work.tile([128, N], F32, tag="iotajf")
            nc.vector.tensor_scalar(iotaj_f[:, :], iota_f[:, :], 2.0, None,
                                    ALU.add)
            nc.vector.tensor_copy(iotaj16[:, :], iotaj_f[:, :])
            # per-column scales 2^l for the view encode: [128, 30]
            sc30 = cpool.tile([128, DEG * 3], F32)
            for l in range(DEG):
                nc.vector.memset(sc30[:, 3 * l:3 * l + 3], float(2.0 ** l))

            for t in range(NT):
                r0 = t * 128
                bins_t = io.tile_from(bins_h[r0:r0 + 128, :])
                w_t = io.tile_from(w_h[r0:r0 + 128, :])
                ur_t = io.tile_from(ur_h[r0:r0 + 128, :])
                org_t = io.tile_from(org_h[r0:r0 + 128, :])
                dir_t = io.tile_from(dir_h[r0:r0 + 128, :])

                # ---- view encode, batched over degs: vt [128, 60] ----
                vt = work.tile([128, 2 * DEG * 3], F32, tag="vt")
                zd = work.tile([128, DEG * 3], F32, tag="zd")
                dir_b = dir_t[:, :].unsqueeze(1).broadcast_to((128, DEG, 3))
                zd3 = zd[:, :].rearrange("p (l k) -> p l k", k=3)
                sc30_3 = sc30[:, :].rearrange("p (l k) -> p l k", k=3)
                nc.vector.tensor_tensor(zd3, dir_b, sc30_3, ALU.mult)
                tv = work.tile([128, DEG * 3], F32, tag="tv")
                nc.vector.tensor_scalar(tv[:, :], zd[:, :], INV_2PI, MAGIC,
                                        ALU.mult, ALU.add)
                nc.vector.tensor_scalar(tv[:, :], tv[:, :], MAGIC, None,
                                        ALU.subtract)
                rv = work.tile([128, DEG * 3], F32, tag="rv")
                nc.vector.scalar_tensor_tensor(rv[:, :], tv[:, :], -TWO_PI,
                                               zd[:, :], ALU.mult, ALU.add)
                nc.scalar.activation(vt[:, 0:DEG * 3], rv[:, :], ACTF.Sin,
                                     bias=0.0, scale=1.0)
                rvc = work.tile([128, DEG * 3], F32, tag="rvc")
                nc.vector.add_range_wrap(rvc[:, :], rv[:, :], TWO_PI / 4.0,
                                         TWO_PI / 2.0, TWO_PI)
                nc.scalar.activation(vt[:, DEG * 3:2 * DEG * 3], rvc[:, :],
                                     ACTF.Sin, bias=0.0, scale=1.0)

                # ---- pdf / cdf  (matches reference op order) ----
                wsum = work.tile([128, 1], F32, tag="wsum")
                nc.vector.tensor_reduce(wsum[:, :], w_t[:, 0:N - 1],
                                        axis=mybir.AxisListType.X, op=ALU.add)
                pad = work.tile([128, 1], F32, tag="pad")
                nc.vector.tensor_scalar(pad[:, :], wsum[:, :], -1.0, EPS,
                                        ALU.mult, ALU.add)
                nc.vector.tensor_scalar(pad[:, :], pad[:, :], 0.0, None, ALU.max)
                wsum2 = work.tile([128, 1], F32, tag="wsum2")
                nc.vector.tensor_tensor(wsum2[:, :], wsum[:, :], pad[:, :], ALU.add)
                rws = work.tile([128, 1], F32, tag="rws")
                nc.vector.reciprocal(rws[:, :], wsum2[:, :])
                padc = work.tile([128, 1], F32, tag="padc")
                nc.vector.tensor_scalar(padc[:, :], pad[:, :], 1.0 / (N - 1), None,
                                        ALU.mult)
                pdf = work.tile([128, N - 1], F32, tag="pdf")
                nc.vector.scalar_tensor_tensor(
                    pdf[:, :], w_t[:, 0:N - 1], padc[:, 0:1],
                    rws[:, 0:1].broadcast_to((128, N - 1)), ALU.add, ALU.mult)

                cdf = work.tile([128, N], F32, tag="cdf")
                nc.vector.memset(cdf[:, 0:1], 0.0)
                nc.vector.memset(cdf[:, N - 1:N], 1.0)
                cs = work.tile([128, N - 2], F32, tag="cs")
                nc.vector.tensor_tensor_scan(cs[:, :], ones_t[:, 0:N - 2],
                                             pdf[:, 0:N - 2], 0.0,
                                             ALU.mult, ALU.add)
                nc.vector.tensor_scalar(cdf[:, 1:N - 1], cs[:, :], 1.0, None,
                                        ALU.min)

                # ---- per-interval slope/intercept (j = 0..126) ----
                d0 = work.tile([128, N - 1], F32, tag="d0")
                nc.vector.tensor_tensor(d0[:, :], cdf[:, 1:N], cdf[:, 0:N - 1],
                                        ALU.subtract)
                db = work.tile([128, N - 1], F32, tag="db")
                nc.vector.tensor_tensor(db[:, :], bins_t[:, 1:N],
                                        bins_t[:, 0:N - 1], ALU.subtract)
                maskE = work.tile([128, N - 1], mybir.dt.uint8, tag="maskE")
                nc.vector.tensor_scalar(maskE[:, :], d0[:, :], EPS, None,
                                        ALU.is_lt)
                dsafe = work.tile([128, N - 1], F32, tag="dsafe")
                nc.vector.select(dsafe[:, :], maskE[:, :], ones_t[:, 0:N - 1],
                                 d0[:, :])
                # M = db / dsafe ; Mstar = M / S ; A = bins - cdf * M
                rdsafe = work.tile([128, N - 1], F32, tag="rdsafe")
                nc.vector.reciprocal(rdsafe[:, :], dsafe[:, :])
                m_t = work.tile([128, N - 1], F32, tag="m_t")
                nc.vector.tensor_tensor(m_t[:, :], db[:, :], rdsafe[:, :],
                                        ALU.mult)
                ms_t = work.tile([128, N], F32, tag="ms_t")
                nc.vector.tensor_scalar(ms_t[:, 0:N - 1], m_t[:, :], 1.0 / S,
                                        None, ALU.mult)
                nc.vector.memset(ms_t[:, N - 1:N], 0.0)
                cm = work.tile([128, N - 1], F32, tag="cm")
                nc.vector.tensor_tensor(cm[:, :], cdf[:, 0:N - 1], m_t[:, :],
                                        ALU.mult)
                a_t = work.tile([128, N], F32, tag="a_t")
                nc.vector.tensor_tensor(a_t[:, 0:N - 1], bins_t[:, 0:N - 1],
                                        cm[:, :], ALU.subtract)
                nc.vector.memset(a_t[:, N - 1:N], 0.0)

                # ---- compare inputs ----
                su = work.tile([128, S], F32, tag="su")
                nc.vector.tensor_tensor(su[:, :], iota_f[:, :], ur_t[:, :], ALU.add)
                y2 = work.tile([128, N - 1], F32, tag="y2")
                nc.vector.tensor_scalar(y2[:, :], cdf[:, 1:N], float(S), None,
                                        ALU.mult)

                # ---- O(N+S) ranks, no dense compare:
                #   r2[j] = #{s : su_s < y_j} = c_j + [su[c_j] < y_j],
                #   c_j = min(floor(y_j), 127).  su[c_j] is a scatter-scan
                #   gather: the inverse of sorted-int c needs no compare.
                cr = work.tile([128, N - 1], F32, tag="cr")
                nc.vector.tensor_scalar(cr[:, :], y2[:, :], 1.0, MAGIC,
                                        ALU.mult, ALU.add)
                nc.vector.tensor_scalar(cr[:, :], cr[:, :], MAGIC, None,
                                        ALU.subtract)
                cgt = work.tile([128, N - 1], F32, tag="cgt")
                nc.vector.tensor_tensor(cgt[:, :], cr[:, :], y2[:, :], ALU.is_gt)
                cfl = work.tile([128, N - 1], F32, tag="cfl")
                nc.vector.tensor_tensor(cfl[:, :], cr[:, :], cgt[:, :],
                                        ALU.subtract)
                nc.vector.tensor_scalar(cfl[:, :], cfl[:, :], float(S - 1),
                                        None, ALU.min)
                # D-scatter: place (j+1) at cell c_j, keep largest j per value
                kdx = work.tile([128, N - 1], F32, tag="kdx")
                nc.vector.tensor_tensor(kdx[:, 0:N - 2], cfl[:, 0:N - 2],
                                        cfl[:, 1:N - 1], ALU.is_lt)
                nc.vector.memset(kdx[:, N - 2:N - 1], 1.0)
                kt1 = work.tile([128, N - 1], F32, tag="kt1")
                nc.vector.tensor_tensor(kt1[:, :], cfl[:, :], kdx[:, :],
                                        ALU.mult)
                kt2 = work.tile([128, N - 1], F32, tag="kt2")
                nc.vector.tensor_scalar(kt2[:, :], kdx[:, :], 1.0, None,
                                        ALU.subtract)
                idxcf = work.tile([128, N - 1], F32, tag="idxcf")
                nc.vector.tensor_tensor(idxcf[:, :], kt1[:, :], kt2[:, :],
                                        ALU.add)
                idxc16 = work.tile([128, N], I16, tag="idxc16")
                nc.vector.tensor_copy(idxc16[:, 0:N - 1], idxcf[:, :])
                nc.vector.memset(idxc16[:, N - 1:N], -1)
                d16 = work.tile([128, S], I16, tag="d16")
                nc.gpsimd.local_scatter(d16[:, :], iotaj16[:, :], idxc16[:, :],
                                        channels=128, num_elems=S, num_idxs=N)
                # F[v] = #{j : c_j <= v} by fill-forward; q'_v = F[v-1]
                aD = work.tile([128, S], F32, tag="aD")
                nc.vector.tensor_scalar(aD[:, :], d16[:, :], 0.0, None,
                                        ALU.is_equal)
                fq = work.tile([128, S + 1], F32, tag="fq")
                nc.vector.memset(fq[:, 0:1], 0.0)
                nc.vector.tensor_tensor_scan(fq[:, 1:S + 1], aD[:, :],
                                             d16[:, :], 0.0, ALU.mult, ALU.add)
                pv = work.tile([128, S], F32, tag="pv")
                nc.vector.tensor_scalar(pv[:, :], d16[:, :], 0.0, None,
                                        ALU.is_gt)
                wt1 = work.tile([128, S], F32, tag="wt1")
                nc.vector.tensor_tensor(wt1[:, :], fq[:, 0:S], pv[:, :],
                                        ALU.mult)
                idxwf = work.tile([128, S], F32, tag="idxwf")
                nc.vector.tensor_scalar(idxwf[:, :], wt1[:, :], 1.0, None,
                                        ALU.subtract)
                idxw16 = work.tile([128, S], I16, tag="idxw16")
                nc.vector.tensor_copy(idxw16[:, :], idxwf[:, :])
                occw = work.tile([128, S], I16, tag="occw")
                nc.gpsimd.local_scatter(occw[:, :], ones16[:, :], idxw16[:, :],
                                        channels=128, num_elems=S, num_idxs=S)
                aW = work.tile([128, S], F32, tag="aW")
                nc.vector.tensor_scalar(aW[:, :], occw[:, :], -1.0, 1.0,
                                        ALU.mult, ALU.add)
                # gather su[c_j] into W cells via int16-halves scatter + scan
                su16 = su[:, :].bitcast(I16).rearrange(
                    "p (s two) -> p s two", two=2)
                w16 = work.tile([128, 2 * S], I16, tag="w16")
                w16v = w16[:, :].rearrange("p (j two) -> p j two", two=2)
                for half in range(2):
                    shalf = work.tile([128, S], I16, tag=f"suh{half}")
                    nc.vector.tensor_copy(shalf[:, :], su16[:, :, half])
                    wsct = work.tile([128, S], I16, tag=f"wsct{half}")
                    nc.gpsimd.local_scatter(wsct[:, :], shalf[:, :],
                                            idxw16[:, :], channels=128,
                                            num_elems=S, num_idxs=S)
                    nc.vector.tensor_tensor_scan(w16v[:, :, half], aW[:, :],
                                                 wsct[:, :], 0.0, ALU.mult,
                                                 ALU.add)
                wg = w16[:, :].bitcast(F32)
                # t_j = [su[c_j] < y_j];  r2[j] = c_j + t_j
                tj = work.tile([128, N - 1], F32, tag="tj")
                nc.vector.tensor_tensor(tj[:, :], wg[:, 0:N - 1], y2[:, :],
                                        ALU.is_lt)
                r2 = work.tile([128, N], F32, tag="r2")
                nc.vector.memset(r2[:, 0:1], 0.0)
                nc.vector.tensor_tensor(r2[:, 1:N], cfl[:, :], tj[:, :],
                                        ALU.add)

                # scatter position for interval j (j = 0..126), keeping only
                # the largest j at each rank: idx_j = r2[j] iff r2[j] < r2[j+1]
                km = work.tile([128, N - 1], F32, tag="km")
                nc.vector.tensor_tensor(km[:, :], r2[:, 0:N - 1], r2[:, 1:N],
                                        ALU.is_lt)
                kt = work.tile([128, N - 1], F32, tag="kt")
                nc.vector.tensor_tensor(kt[:, :], r2[:, 0:N - 1], km[:, :],
                                        ALU.mult)
                km1 = work.tile([128, N - 1], F32, tag="km1")
                nc.vector.tensor_scalar(km1[:, :], km[:, :], 1.0, None,
                                        ALU.subtract)
                idxf = work.tile([128, N - 1], F32, tag="idxf")
                nc.vector.tensor_tensor(idxf[:, :], kt[:, :], km1[:, :], ALU.add)
                idx16 = work.tile([128, N], I16, tag="idx16")
                nc.vector.tensor_copy(idx16[:, 0:N - 1], idxf[:, :])
                nc.vector.memset(idx16[:, N - 1:N], -1)

                # occupancy scatter + fill-forward mask
                occ16 = work.tile([128, S], I16, tag="occ16")
                nc.gpsimd.local_scatter(occ16[:, :], ones16[:, :], idx16[:, :],
                                        channels=128, num_elems=S, num_idxs=N)
                amask = work.tile([128, S], I16, tag="amask")
                nc.vector.tensor_scalar(amask[:, :], occ16[:, :], -1.0, 1.0,
                                        ALU.mult, ALU.add)

                # exact f32 gathers A[k_s], Mstar[k_s]: scatter the two int16
                # halves of each value, fill-forward scan, reinterleave
                gath = {}
                for name, vsrc in (("A", a_t), ("M", ms_t)):
                    v16 = vsrc[:, :].bitcast(I16).rearrange(
                        "p (j two) -> p j two", two=2)
                    g16 = work.tile([128, 2 * S], I16, tag=f"g16{name}")
                    g16v = g16[:, :].rearrange("p (s two) -> p s two", two=2)
                    for half in range(2):
                        hsrc = work.tile([128, N], I16, tag=f"h{name}{half}")
                        nc.vector.tensor_copy(hsrc[:, :], v16[:, :, half])
                        sct = work.tile([128, S], I16, tag=f"sct{name}{half}")
                        nc.gpsimd.local_scatter(sct[:, :], hsrc[:, :],
                                                idx16[:, :], channels=128,
                                                num_elems=S, num_idxs=N)
                        nc.vector.tensor_tensor_scan(g16v[:, :, half],
                                                     amask[:, :], sct[:, :],
                                                     0.0, ALU.mult, ALU.add)
                    gath[name] = g16[:, :].bitcast(F32)

                # ---- interpolation: smp = su*Mstar[k] + A[k] ----
                tmp = work.tile([128, S], F32, tag="tmp")
                nc.vector.tensor_tensor(tmp[:, :], su[:, :], gath["M"], ALU.mult)
                smp = work.tile([128, S], F32, tag="smp")
                nc.vector.tensor_tensor(smp[:, :], tmp[:, :], gath["A"], ALU.add)

                # ---- points, s-major interleaved [128, S*3] ----
                pts = work.tile([128, 3 * S], F32, tag="pts")
                pts_k = pts[:, :].rearrange("p (s k) -> p k s", k=3)
                for k in range(3):
                    nc.vector.scalar_tensor_tensor(
                        pts_k[:, k, :], smp[:, :], dir_t[:, k:k + 1],
                        org_t[:, k:k + 1].broadcast_to((128, S)),
                        ALU.mult, ALU.add)
                yb = work.tile([128, 3 * S], F32, tag="yb")
                nc.vector.tensor_scalar(yb[:, :], pts[:, :], INV_2PI, None,
                                        ALU.mult)

                # ---- positional encodes + staging + store, per s-half ----
                stg = stage_pool.tile([128, S * CH], F32, tag="stg")
                stg3 = stg[:, :].rearrange("p (s c) -> p s c", c=CH)
                # range-reduce once at deg 0 (magic round + Cody-Waite),
                # then halve the range per degree with one wrap each
                t1 = work.tile([128, 3 * S], F32, tag="t1")
                nc.vector.tensor_scalar(t1[:, :], yb[:, :], 1.0, MAGIC,
                                        ALU.mult, ALU.add)
                nc.vector.tensor_scalar(t1[:, :], t1[:, :], MAGIC, None,
                                        ALU.subtract)
                rs = work.tile([128, 3 * S], F32, tag="rs")
                nc.vector.cody_waite_cascade(rs[:, :], pts[:, :], t1[:, :],
                                             CW1, CW2, CW3)
                for l in range(DEG):
                    if l > 0:
                        sc = float(2.0 ** l)
                        rs_new = work.tile([128, 3 * S], F32, tag="rs")
                        nc.vector.add_range_wrap(rs_new[:, :], rs[:, :], 0.0,
                                                 (TWO_PI / 2.0) / sc,
                                                 TWO_PI / sc)
                        rs = rs_new
                    _emit_encode_deg(nc, stg3, rs, l, work)
                # view block: broadcast [128, 60] over all s
                vin = vt[:, :].unsqueeze(1).broadcast_to((128, S, 60))
                nc.scalar.copy(stg3[:, :, 60:120], vin)
                nc.sync.dma_start(out_h[r0:r0 + 128, :], stg[:, :])
    return nc


_NC_CACHE = {}


def _get_nc():
    if "nc" not in _NC_CACHE:
        nc = bacc.Bacc('TRN2', target_bir_lowering=False)
        _emit_core_kernel(nc)
        nc.compile()
        _NC_CACHE["nc"] = nc
    return _NC_CACHE["nc"]


def _shard(inputs):
    in_maps = []
    for c in range(NCORES):
        sl = slice(c * RC, (c + 1) * RC)
        in_maps.append({
            "origins": np.ascontiguousarray(inputs["origins"][sl]),
            "directions": np.ascontiguousarray(inputs["directions"][sl]),
            "bins": np.ascontiguousarray(inputs["bins"][sl]),
            "weights": np.ascontiguousarray(inputs["weights"][sl]),
            "u_rand": np.ascontiguousarray(inputs["u_rand"][sl]),
        })
    return in_maps


LAST_EXEC_NS = None
LAST_TRACE_PATH = None
LAST_RES = None


def kernel(**inputs):
    global LAST_EXEC_NS, LAST_TRACE_PATH, LAST_RES
    from concourse.bass_utils import run_bass_kernel_spmd
    nc = _get_nc()
    in_maps = _shard(inputs)
    trace = bool(os.environ.get("BASS_TRACE"))
    res = run_bass_kernel_spmd(nc, in_maps, core_ids=list(range(NCORES)),
                               trace=trace)
    if trace:
        LAST_RES = res
        LAST_EXEC_NS = res.exec_time_ns
        print("HW exec_time_ns:", res.exec_time_ns,
              "mean:", res.mean_exec_time_ns)
        if res.instructions_and_trace:
            LAST_TRACE_PATH = res.instructions_and_trace[1]
            print("trace path:", res.instructions_and_trace[1])
    parts = [res.results[c]["out"].reshape(RC, S, CH) for c in range(NCORES)]
    return np.concatenate(parts, axis=0).astype(np.float32)


def simulate_one_core(core_inputs):
    """CoreSim path for numerics debugging (no hardware)."""
    from concourse.bass_interp import CoreSim
    nc = bacc.Bacc('TRN2', target_bir_lowering=False)
    _emit_core_kernel(nc)
    nc.compile()
    sim = CoreSim(nc, require_finite=False, require_nnan=False)
    if sim.instruction_executor is not None:
        sim.instruction_executor.ignore_data_errors = True
    for k, v in core_inputs.items():
        sim.tensor(k)[:] = v
    sim.simulate()
    return np.array(sim.tensor("out")).reshape(RC, S, CH)


# revision 50
# speedup vs baseline: 1.0235x; 1.0017x over previous
"""NeRF hierarchical sampling + positional encoding kernel for Trainium2.

Full inputs -> shard rays across 8 cores (data-parallel over the ray axis)
-> one Bass program per core (8 tiles of 128 rays) -> full output.

O(N+S) per-ray sampler (no dense S*N compare), fp32-exact vs reference:
  pdf/cdf prep, then per-interval slope/intercept:
    Mstar_j = (bins_{j+1}-bins_j)/denomsafe_j/S,  A_j = bins_j - cdf_j*M_j
  so sample_s = su_s*Mstar[k_s] + A[k_s] with su_s = s + u_rand, and
  k_s = searchsorted(cdf, u)-1.
  Ranks r_j = #{s: su_s < S*cdf_j} come from r_j = c_j + [su[c_j] < y_j]
  (c_j = clamped floor of y_j = S*cdf_j): su[c_j] is gathered by gpsimd
  local_scatter (per-partition indices) + DVE fill-forward scans; the
  inverse of the sorted integer sequence c is itself one scatter + scan.
  A[k]/Mstar[k] are gathered the same way, scattering each fp32 value as
  two int16 bit-halves (exact) at deduplicated rank positions.
  Positional encoding: one magic-round + Cody-Waite range reduction at
  deg 0, then one add_range_wrap halving per degree; sin and cos(=shifted
  sin) evaluated by the scalar engine's Sin activation writing straight
  into the staging tile; view encode batched over all degrees per tile.
Engines: DVE does the vector work, gpsimd only local_scatters (library 7),
ACT does all transcendentals + the broadcast view-block copy, sync issues
contiguous DMAs (output is DMA-bandwidth-floor ~154us/core).
"""

import os
import sys

for _p in ("/opt/trn_rl_repo", "/root/.axon_site/_ro/trn_rl_repo"):
    if os.path.isdir(_p) and _p not in sys.path:
        sys.path.insert(0, _p)

import numpy as np

import concourse.bass as bass
import concourse.bacc as bacc
import concourse.mybir as mybir
from concourse import tile

F32 = mybir.dt.float32
BF16 = mybir.dt.bfloat16
I32 = mybir.dt.int32
I16 = mybir.dt.int16
ALU = mybir.AluOpType
ACTF = mybir.ActivationFunctionType

R, N, S = 8192, 128, 128
NCORES = 8
RC = R // NCORES          # rays per core
NT = RC // 128            # ray tiles per core (128 rays each)
DEG = 10
EPS = 1e-5
CH = 120                  # output channels per sample
OUTW = S * CH             # flattened output row per ray

PI = float(np.float32(np.pi))
TWO_PI = 6.283185307179586
INV_2PI = float(np.float32(1.0 / TWO_PI))
MAGIC = float(np.float32(1.5 * 2**23))  # round-to-int magic constant
# Cody-Waite split of 2*pi (fallback encode path)
CW1 = 6.25
CW2 = 0.033203125
CW3 = float(np.float32(TWO_PI - CW1 - CW2))

HALF_S = 64               # encode/staging processed in s-halves
BIG = 1.0e9               # pad sentinel for compare columns

def _emit_encode_deg(nc, stg3, rs, l, work):
    """Emit sin/cos for degree l from range-reduced rs (s-major [128, 3*S]).

    rs holds x - k*2pi/2^l with |rs| <= pi/2^l; ACT applies scale 2^l.
    """
    sc = float(2.0 ** l)
    rs_3 = rs[:, :].rearrange("p (s k) -> p s k", k=3)
    sin_dst = stg3[:, :, 3 * l:3 * l + 3]
    nc.scalar.activation(sin_dst, rs_3, ACTF.Sin, bias=0.0, scale=sc)
    ws = work.tile([128, 3 * S], F32, tag="ws")
    nc.vector.add_range_wrap(ws[:, :], rs[:, :], (TWO_PI / 4.0) / sc,
                             (TWO_PI / 2.0) / sc, TWO_PI / sc)
    ws_3 = ws[:, :].rearrange("p (s k) -> p s k", k=3)
    cos_dst = stg3[:, :, 30 + 3 * l:30 + 3 * l + 3]
    nc.scalar.activation(cos_dst, ws_3, ACTF.Sin, bias=0.0, scale=sc)


def _emit_core_kernel(nc):
    """Emit the whole per-core program under a TileContext."""
    org_h = nc.dram_tensor("origins", [RC, 3], F32, kind="ExternalInput")
    dir_h = nc.dram_tensor("directions", [RC, 3], F32, kind="ExternalInput")
    bins_h = nc.dram_tensor("bins", [RC, N], F32, kind="ExternalInput")
    w_h = nc.dram_tensor("weights", [RC, N], F32, kind="ExternalInput")
    ur_h = nc.dram_tensor("u_rand", [RC, S], F32, kind="ExternalInput")
    out_h = nc.dram_tensor("out", [RC, OUTW], F32, kind="ExternalOutput")

    with tile.TileContext(nc) as tc:
        with (
            tc.tile_pool(name="io", bufs=3) as io,
            tc.tile_pool(name="cmp", bufs=2) as cmp_pool,
            tc.tile_pool(name="stage", bufs=2) as stage_pool,
            tc.tile_pool(name="work", bufs=2) as work,
            tc.tile_pool(name="const", bufs=1) as cpool,
        ):
            # --- constants (once) ---
            iota_i = cpool.tile([128, S], I32)
            nc.gpsimd.iota(iota_i[:, :], pattern=[[1, S]], base=0,
                           channel_multiplier=0)
            # all later gpsimd work is local_scatter (library 7)
            from concourse import library_config
            nc.gpsimd.load_library(library_config.local_scatter)
            iota_f = cpool.tile([128, S], F32)
            nc.vector.tensor_copy(iota_f[:, :], iota_i[:, :])
            ones_t = cpool.tile([128, S], F32)
            nc.vector.memset(ones_t[:, :], 1.0)
            ones16 = cpool.tile([128, N], I16)
            nc.vector.memset(ones16[:, :], 1)
            # iotaj16[j] = j+2 (int16) D-scatter payload: the fill-forward
            # then yields q'_v + 1 directly, so idxw = fq*pv - 1
            iotaj16 = cpool.tile([128, N], I16)
            iotaj_f = work.tile([128, N], F32, tag="iotajf")
            nc.vector.tensor_scalar(iotaj_f[:, :], iota_f[:, :], 2.0, None,
                                    ALU.add)
            nc.vector.tensor_copy(iotaj16[:, :], iotaj_f[:, :])
            # per-column scales 2^l for the view encode: [128, 30]
            sc30 = cpool.tile([128, DEG * 3], F32)
            for l in range(DEG):
                nc.vector.memset(sc30[:, 3 * l:3 * l + 3], float(2.0 ** l))

            for t in range(NT):
                r0 = t * 128
                bins_t = io.tile_from(bins_h[r0:r0 + 128, :])
                w_t = io.tile_from(w_h[r0:r0 + 128, :])
                ur_t = io.tile_from(ur_h[r0:r0 + 128, :])
                org_t = io.tile_from(org_h[r0:r0 + 128, :])
                dir_t = io.tile_from(dir_h[r0:r0 + 128, :])

                # ---- view encode, batched over degs: vt [128, 60] ----
                vt = work.tile([128, 2 * DEG * 3], F32, tag="vt")
                zd = work.tile([128, DEG * 3], F32, tag="zd")
                dir_b = dir_t[:, :].unsqueeze(1).broadcast_to((128, DEG, 3))
                zd3 = zd[:, :].rearrange("p (l k) -> p l k", k=3)
                sc30_3 = sc30[:, :].rearrange("p (l k) -> p l k", k=3)
                nc.vector.tensor_tensor(zd3, dir_b, sc30_3, ALU.mult)
                tv = work.tile([128, DEG * 3], F32, tag="tv")
                nc.vector.tensor_scalar(tv[:, :], zd[:, :], INV_2PI, MAGIC,
                                        ALU.mult, ALU.add)
                nc.vector.tensor_scalar(tv[:, :], tv[:, :], MAGIC, None,
                                        ALU.subtract)
                rv = work.tile([128, DEG * 3], F32, tag="rv")
                nc.vector.scalar_tensor_tensor(rv[:, :], tv[:, :], -TWO_PI,
                                               zd[:, :], ALU.mult, ALU.add)
                nc.scalar.activation(vt[:, 0:DEG * 3], rv[:, :], ACTF.Sin,
                                     bias=0.0, scale=1.0)
                rvc = work.tile([128, DEG * 3], F32, tag="rvc")
                nc.vector.add_range_wrap(rvc[:, :], rv[:, :], TWO_PI / 4.0,
                                         TWO_PI / 2.0, TWO_PI)
                nc.scalar.activation(vt[:, DEG * 3:2 * DEG * 3], rvc[:, :],
                                     ACTF.Sin, bias=0.0, scale=1.0)

                # su depends only on u_rand: compute it (and the int16
                # halves the W-gather scatters need) before the cdf chain
                su = work.tile([128, S], F32, tag="su")
                nc.vector.tensor_tensor(su[:, :], iota_f[:, :], ur_t[:, :], ALU.add)
                su16e = su[:, :].bitcast(I16).rearrange(
                    "p (s two) -> p s two", two=2)
                suh = []
                for half in range(2):
                    sh = work.tile([128, S], I16, tag=f"suh{half}")
                    nc.vector.tensor_copy(sh[:, :], su16e[:, :, half])
                    suh.append(sh)

                # ---- pdf / cdf  (matches reference op order) ----
                wsum = work.tile([128, 1], F32, tag="wsum")
                nc.vector.tensor_reduce(wsum[:, :], w_t[:, 0:N - 1],
                                        axis=mybir.AxisListType.X, op=ALU.add)
                pad = work.tile([128, 1], F32, tag="pad")
                nc.vector.tensor_scalar(pad[:, :], wsum[:, :], -1.0, EPS,
                                        ALU.mult, ALU.add)
                nc.vector.tensor_scalar(pad[:, :], pad[:, :], 0.0, None, ALU.max)
                wsum2 = work.tile([128, 1], F32, tag="wsum2")
                nc.vector.tensor_tensor(wsum2[:, :], wsum[:, :], pad[:, :], ALU.add)
                rws = work.tile([128, 1], F32, tag="rws")
                nc.vector.reciprocal(rws[:, :], wsum2[:, :])
                padc = work.tile([128, 1], F32, tag="padc")
                nc.vector.tensor_scalar(padc[:, :], pad[:, :], 1.0 / (N - 1), None,
                                        ALU.mult)
                pdf = work.tile([128, N - 1], F32, tag="pdf")
                nc.vector.scalar_tensor_tensor(
                    pdf[:, :], w_t[:, 0:N - 1], padc[:, 0:1],
                    rws[:, 0:1].broadcast_to((128, N - 1)), ALU.add, ALU.mult)

                cdf = work.tile([128, N], F32, tag="cdf")
                nc.vector.memset(cdf[:, 0:1], 0.0)
                nc.vector.memset(cdf[:, N - 1:N], 1.0)
                cs = work.tile([128, N - 2], F32, tag="cs")
                nc.vector.tensor_tensor_scan(cs[:, :], ones_t[:, 0:N - 2],
                                             pdf[:, 0:N - 2], 0.0,
                                             ALU.mult, ALU.add)
                nc.vector.tensor_scalar(cdf[:, 1:N - 1], cs[:, :], 1.0, None,
                                        ALU.min)

                # ---- per-interval slope/intercept (j = 0..126) ----
                d0 = work.tile([128, N - 1], F32, tag="d0")
                nc.vector.tensor_tensor(d0[:, :], cdf[:, 1:N], cdf[:, 0:N - 1],
                                        ALU.subtract)
                db = work.tile([128, N - 1], F32, tag="db")
                nc.vector.tensor_tensor(db[:, :], bins_t[:, 1:N],
                                        bins_t[:, 0:N - 1], ALU.subtract)
                maskE = work.tile([128, N - 1], mybir.dt.uint8, tag="maskE")
                nc.vector.tensor_scalar(maskE[:, :], d0[:, :], EPS, None,
                                        ALU.is_lt)
                dsafe = work.tile([128, N - 1], F32, tag="dsafe")
                nc.vector.select(dsafe[:, :], maskE[:, :], ones_t[:, 0:N - 1],
                                 d0[:, :])
                # M = db / dsafe ; Mstar = M / S ; A = bins - cdf * M
                rdsafe = work.tile([128, N - 1], F32, tag="rdsafe")
                nc.vector.reciprocal(rdsafe[:, :], dsafe[:, :])
                m_t = work.tile([128, N - 1], F32, tag="m_t")
                nc.vector.tensor_tensor(m_t[:, :], db[:, :], rdsafe[:, :],
                                        ALU.mult)
                ms_t = work.tile([128, N], F32, tag="ms_t")
                nc.vector.tensor_scalar(ms_t[:, 0:N - 1], m_t[:, :], 1.0 / S,
                                        None, ALU.mult)
                nc.vector.memset(ms_t[:, N - 1:N], 0.0)
                cm = work.tile([128, N - 1], F32, tag="cm")
                nc.vector.tensor_tensor(cm[:, :], cdf[:, 0:N - 1], m_t[:, :],
                                        ALU.mult)
                a_t = work.tile([128, N], F32, tag="a_t")
                nc.vector.tensor_tensor(a_t[:, 0:N - 1], bins_t[:, 0:N - 1],
                                        cm[:, :], ALU.subtract)
                nc.vector.memset(a_t[:, N - 1:N], 0.0)

                y2 = work.tile([128, N - 1], F32, tag="y2")
                nc.vector.tensor_scalar(y2[:, :], cdf[:, 1:N], float(S), None,
                                        ALU.mult)

                # ---- O(N+S) ranks, no dense compare:
                #   r2[j] = #{s : su_s < y_j} = c_j + [su[c_j] < y_j],
                #   c_j = min(floor(y_j), 127).  su[c_j] is a scatter-scan
                #   gather: the inverse of sorted-int c needs no compare.
                cr = work.tile([128, N - 1], F32, tag="cr")
                nc.vector.tensor_scalar(cr[:, :], y2[:, :], 1.0, MAGIC,
                                        ALU.mult, ALU.add)
                nc.vector.tensor_scalar(cr[:, :], cr[:, :], MAGIC, None,
                                        ALU.subtract)
                cgt = work.tile([128, N - 1], F32, tag="cgt")
                nc.vector.tensor_tensor(cgt[:, :], cr[:, :], y2[:, :], ALU.is_gt)
                cfl = work.tile([128, N - 1], F32, tag="cfl")
                nc.vector.tensor_tensor(cfl[:, :], cr[:, :], cgt[:, :],
                                        ALU.subtract)
                nc.vector.tensor_scalar(cfl[:, :], cfl[:, :], float(S - 1),
                                        None, ALU.min)
                # D-scatter: place (j+1) at cell c_j, keep largest j per value
                kdx = work.tile([128, N - 1], F32, tag="kdx")
                nc.vector.tensor_tensor(kdx[:, 0:N - 2], cfl[:, 0:N - 2],
                                        cfl[:, 1:N - 1], ALU.is_lt)
                nc.vector.memset(kdx[:, N - 2:N - 1], 1.0)
                kt1 = work.tile([128, N - 1], F32, tag="kt1")
                nc.vector.tensor_tensor(kt1[:, :], cfl[:, :], kdx[:, :],
                                        ALU.mult)
                kt2 = work.tile([128, N - 1], F32, tag="kt2")
                nc.vector.tensor_scalar(kt2[:, :], kdx[:, :], 1.0, None,
                                        ALU.subtract)
                idxcf = work.tile([128, N - 1], F32, tag="idxcf")
                nc.vector.tensor_tensor(idxcf[:, :], kt1[:, :], kt2[:, :],
                                        ALU.add)
                idxc16 = work.tile([128, N], I16, tag="idxc16")
                nc.vector.tensor_copy(idxc16[:, 0:N - 1], idxcf[:, :])
                nc.vector.memset(idxc16[:, N - 1:N], -1)
                d16 = work.tile([128, S], I16, tag="d16")
                nc.gpsimd.local_scatter(d16[:, :], iotaj16[:, :], idxc16[:, :],
                                        channels=128, num_elems=S, num_idxs=N)
                # F[v] = #{j : c_j <= v} by fill-forward; q'_v = F[v-1]
                aD = work.tile([128, S], F32, tag="aD")
                nc.vector.tensor_scalar(aD[:, :], d16[:, :], 0.0, None,
                                        ALU.is_equal)
                fq = work.tile([128, S + 1], F32, tag="fq")
                nc.vector.memset(fq[:, 0:1], 0.0)
                nc.vector.tensor_tensor_scan(fq[:, 1:S + 1], aD[:, :],
                                             d16[:, :], 0.0, ALU.mult, ALU.add)
                pv = work.tile([128, S], F32, tag="pv")
                nc.vector.tensor_scalar(pv[:, :], d16[:, :], 0.0, None,
                                        ALU.is_gt)
                wt1 = work.tile([128, S], F32, tag="wt1")
                nc.vector.tensor_tensor(wt1[:, :], fq[:, 0:S], pv[:, :],
                                        ALU.mult)
                idxwf = work.tile([128, S], F32, tag="idxwf")
                nc.vector.tensor_scalar(idxwf[:, :], wt1[:, :], 1.0, None,
                                        ALU.subtract)
                idxw16 = work.tile([128, S], I16, tag="idxw16")
                nc.vector.tensor_copy(idxw16[:, :], idxwf[:, :])
                occw = work.tile([128, S], I16, tag="occw")
                nc.gpsimd.local_scatter(occw[:, :], ones16[:, :], idxw16[:, :],
                                        channels=128, num_elems=S, num_idxs=S)
                aW = work.tile([128, S], F32, tag="aW")
                nc.vector.tensor_scalar(aW[:, :], occw[:, :], -1.0, 1.0,
                                        ALU.mult, ALU.add)
                # gather su[c_j] into W cells via int16-halves scatter + scan
                w16 = work.tile([128, 2 * S], I16, tag="w16")
                w16v = w16[:, :].rearrange("p (j two) -> p j two", two=2)
                for half in range(2):
                    wsct = work.tile([128, S], I16, tag=f"wsct{half}")
                    nc.gpsimd.local_scatter(wsct[:, :], suh[half][:, :],
                                            idxw16[:, :], channels=128,
                                            num_elems=S, num_idxs=S)
                    nc.vector.tensor_tensor_scan(w16v[:, :, half], aW[:, :],
                                                 wsct[:, :], 0.0, ALU.mult,
                                                 ALU.add)
                wg = w16[:, :].bitcast(F32)
                # t_j = [su[c_j] < y_j];  r2[j] = c_j + t_j
                tj = work.tile([128, N - 1], F32, tag="tj")
                nc.vector.tensor_tensor(tj[:, :], wg[:, 0:N - 1], y2[:, :],
                                        ALU.is_lt)
                r2 = work.tile([128, N], F32, tag="r2")
                nc.vector.memset(r2[:, 0:1], 0.0)
                nc.vector.tensor_tensor(r2[:, 1:N], cfl[:, :], tj[:, :],
                                        ALU.add)

                # scatter position for interval j (j = 0..126), keeping only
                # the largest j at each rank: idx_j = r2[j] iff r2[j] < r2[j+1]
                km = work.tile([128, N - 1], F32, tag="km")
                nc.vector.tensor_tensor(km[:, :], r2[:, 0:N - 1], r2[:, 1:N],
                                        ALU.is_lt)
                kt = work.tile([128, N - 1], F32, tag="kt")
                nc.vector.tensor_tensor(kt[:, :], r2[:, 0:N - 1], km[:, :],
                                        ALU.mult)
                km1 = work.tile([128, N - 1], F32, tag="km1")
                nc.vector.tensor_scalar(km1[:, :], km[:, :], 1.0, None,
                                        ALU.subtract)
                idxf = work.tile([128, N - 1], F32, tag="idxf")
                nc.vector.tensor_tensor(idxf[:, :], kt[:, :], km1[:, :], ALU.add)
                idx16 = work.tile([128, N], I16, tag="idx16")
                nc.vector.tensor_copy(idx16[:, 0:N - 1], idxf[:, :])
                nc.vector.memset(idx16[:, N - 1:N], -1)

                # occupancy scatter + fill-forward mask
                occ16 = work.tile([128, S], I16, tag="occ16")
                nc.gpsimd.local_scatter(occ16[:, :], ones16[:, :], idx16[:, :],
                                        channels=128, num_elems=S, num_idxs=N)
                amask = work.tile([128, S], I16, tag="amask")
                nc.vector.tensor_scalar(amask[:, :], occ16[:, :], -1.0, 1.0,
                                        ALU.mult, ALU.add)

                # exact f32 gathers A[k_s], Mstar[k_s]: scatter the two int16
                # halves of each value, fill-forward scan, reinterleave
                gath = {}
                for name, vsrc in (("A", a_t), ("M", ms_t)):
                    v16 = vsrc[:, :].bitcast(I16).rearrange(
                        "p (j two) -> p j two", two=2)
                    g16 = work.tile([128, 2 * S], I16, tag=f"g16{name}")
                    g16v = g16[:, :].rearrange("p (s two) -> p s two", two=2)
                    for half in range(2):
                        hsrc = work.tile([128, N], I16, tag=f"h{name}{half}")
                        nc.vector.tensor_copy(hsrc[:, :], v16[:, :, half])
                        sct = work.tile([128, S], I16, tag=f"sct{name}{half}")
                        nc.gpsimd.local_scatter(sct[:, :], hsrc[:, :],
                                                idx16[:, :], channels=128,
                                                num_elems=S, num_idxs=N)
                        nc.vector.tensor_tensor_scan(g16v[:, :, half],
                                                     amask[:, :], sct[:, :],
                                                     0.0, ALU.mult, ALU.add)
                    gath[name] = g16[:, :].bitcast(F32)

                # ---- interpolation: smp = su*Mstar[k] + A[k] ----
                tmp = work.tile([128, S], F32, tag="tmp")
                nc.vector.tensor_tensor(tmp[:, :], su[:, :], gath["M"], ALU.mult)
                smp = work.tile([128, S], F32, tag="smp")
                nc.vector.tensor_tensor(smp[:, :], tmp[:, :], gath["A"], ALU.add)

                # ---- points, s-major interleaved [128, S*3] ----
                pts = work.tile([128, 3 * S], F32, tag="pts")
                pts_k = pts[:, :].rearrange("p (s k) -> p k s", k=3)
                for k in range(3):
                    nc.vector.scalar_tensor_tensor(
                        pts_k[:, k, :], smp[:, :], dir_t[:, k:k + 1],
                        org_t[:, k:k + 1].broadcast_to((128, S)),
                        ALU.mult, ALU.add)
                yb = work.tile([128, 3 * S], F32, tag="yb")
                nc.vector.tensor_scalar(yb[:, :], pts[:, :], INV_2PI, None,
                                        ALU.mult)

                # ---- positional encodes + staging + store, per s-half ----
                stg = stage_pool.tile([128, S * CH], F32, tag="stg")
                stg3 = stg[:, :].rearrange("p (s c) -> p s c", c=CH)
                # range-reduce once at deg 0 (magic round + Cody-Waite),
                # then halve the range per degree with one wrap each
                t1 = work.tile([128, 3 * S], F32, tag="t1")
                nc.vector.tensor_scalar(t1[:, :], yb[:, :], 1.0, MAGIC,
                                        ALU.mult, ALU.add)
                nc.vector.tensor_scalar(t1[:, :], t1[:, :], MAGIC, None,
                                        ALU.subtract)
                rs = work.tile([128, 3 * S], F32, tag="rs")
                nc.vector.cody_waite_cascade(rs[:, :], pts[:, :], t1[:, :],
                                             CW1, CW2, CW3)
                for l in range(DEG):
                    if l > 0:
                        sc = float(2.0 ** l)
                        rs_new = work.tile([128, 3 * S], F32, tag="rs")
                        nc.vector.add_range_wrap(rs_new[:, :], rs[:, :], 0.0,
                                                 (TWO_PI / 2.0) / sc,
                                                 TWO_PI / sc)
                        rs = rs_new
                    _emit_encode_deg(nc, stg3, rs, l, work)
                # view block: broadcast [128, 60] over all s
                vin = vt[:, :].unsqueeze(1).broadcast_to((128, S, 60))
                nc.scalar.copy(stg3[:, :, 60:120], vin)
                nc.sync.dma_start(out_h[r0:r0 + 128, :], stg[:, :])
    return nc


_NC_CACHE = {}


def _get_nc():
    if "nc" not in _NC_CACHE:
        nc = bacc.Bacc('TRN2', target_bir_lowering=False)
        _emit_core_kernel(nc)
        nc.compile()
        _NC_CACHE["nc"] = nc
    return _NC_CACHE["nc"]


def _shard(inputs):
    in_maps = []
    for c in range(NCORES):
        sl = slice(c * RC, (c + 1) * RC)
        in_maps.append({
            "origins": np.ascontiguousarray(inputs["origins"][sl]),
            "directions": np.ascontiguousarray(inputs["directions"][sl]),
            "bins": np.ascontiguousarray(inputs["bins"][sl]),
            "weights": np.ascontiguousarray(inputs["weights"][sl]),
            "u_rand": np.ascontiguousarray(inputs["u_rand"][sl]),
        })
    return in_maps


LAST_EXEC_NS = None
LAST_TRACE_PATH = None
LAST_RES = None


def kernel(**inputs):
    global LAST_EXEC_NS, LAST_TRACE_PATH, LAST_RES
    from concourse.bass_utils import run_bass_kernel_spmd
    nc = _get_nc()
    in_maps = _shard(inputs)
    trace = bool(os.environ.get("BASS_TRACE"))
    res = run_bass_kernel_spmd(nc, in_maps, core_ids=list(range(NCORES)),
                               trace=trace)
    if trace:
        LAST_RES = res
        LAST_EXEC_NS = res.exec_time_ns
        print("HW exec_time_ns:", res.exec_time_ns,
              "mean:", res.mean_exec_time_ns)
        if res.instructions_and_trace:
            LAST_TRACE_PATH = res.instructions_and_trace[1]
            print("trace path:", res.instructions_and_trace[1])
    parts = [res.results[c]["out"].reshape(RC, S, CH) for c in range(NCORES)]
    return np.concatenate(parts, axis=0).astype(np.float32)


def simulate_one_core(core_inputs):
    """CoreSim path for numerics debugging (no hardware)."""
    from concourse.bass_interp import CoreSim
    nc = bacc.Bacc('TRN2', target_bir_lowering=False)
    _emit_core_kernel(nc)
    nc.compile()
    sim = CoreSim(nc, require_finite=False, require_nnan=False)
    if sim.instruction_executor is not None:
        sim.instruction_executor.ignore_data_errors = True
    for k, v in core_inputs.items():
        sim.tensor(k)[:] = v
    sim.simulate()
    return np.array(sim.tensor("out")).reshape(RC, S, CH)
